# revision 6
# baseline (speedup 1.0000x reference)
"""Trainium2 Bass kernel for the LMU problem (nn_LMU_33586644255224).

Strategy: batch-parallel over 8 NeuronCores (4 batches/core). Per core the
LMU scan m_t = m_{t-1} @ A + u_t * Brow (memory order 256) is computed as an
8-step *blocked* scan: 8 independent dependency streams, each advancing by
A^8 per step with the 8 intervening inputs injected through a host-built
augmented weight matrix [A^8; Brow A^7; ...; Brow A^0].  Time is split into
8 chunks of 128 steps whose initial carries come from a tiny hierarchical
scan (per-chunk Legendre sums + sequential A^128 propagation).  Matmuls run
in fp32r (full PE rate at N>=256).  The hidden Dense(512) runs in bf16 as 16
PSUM-accumulated matmuls per (chunk, h-tile) directly over the scan output
layout, with tanh+bias fused on the scalar engine.
"""

import os
import numpy as np
import ml_dtypes

import concourse.bass as bass
import concourse.bacc as bacc
import concourse.tile as tile
from concourse import mybir
from concourse.bass_utils import run_bass_kernel_spmd

F32 = mybir.dt.float32
F32R = mybir.dt.float32r
BF16 = mybir.dt.bfloat16

B_SZ, T, INPUT_D = 32, 1024, 256
D, ORDER, HIDDEN = 8, 256, 512
NCORE = 8
B_L = B_SZ // NCORE          # 4 batches per core
NCH, C, R = 8, 128, 8        # chunks/core, chunk length, scan block size
CBD = NCH * B_L * D          # 256 scan columns (c, b, d)
BD = B_L * D                 # 32


def _host_precompute(A, Bmat, kern, Wh, bh):
    """Weight-derived constants, float64 internally."""
    A64 = A.astype(np.float64)
    Brow = Bmat[0].astype(np.float64)
    Apow = [np.eye(ORDER)]
    for _ in range(C):
        Apow.append(Apow[-1] @ A64)
    P = [Brow @ Apow[k] for k in range(R)]          # P[k] = Brow A^k

    # A8aug (264,256) = [A^8 ; Brow A^7 ; ... ; Brow A^0]
    A8aug = np.concatenate([Apow[R]] + [P[R - 1 - r][None] for r in range(R)], 0)
    # G_j (264,256) = [A^(j+1) ; rows q: Brow A^(j-q) if q<=j else 0]
    Gs = []
    for j in range(R):
        rows = [Apow[j + 1]]
        for q in range(R):
            rows.append(P[j - q][None] if q <= j else np.zeros((1, ORDER)))
        Gs.append(np.concatenate(rows, 0))
    PflipAll = np.stack([Brow @ Apow[C - 1 - i] for i in range(C)], 0)  # (128,256)
    A128 = Apow[C]

    f32 = lambda v: np.ascontiguousarray(v, dtype=np.float32)
    # gk0/gk1: (128, 9*256), col-block g<8 -> G_g, block 8 -> A8aug
    blocks = Gs + [A8aug]
    gk0 = np.concatenate([b[0:128] for b in blocks], 1)
    gk1 = np.concatenate([b[128:256] for b in blocks], 1)
    augsm = np.concatenate([b[256:264] for b in blocks], 1)   # (8, 9*256)
    a128p = np.concatenate([A128[0:128], A128[128:256]], 1)   # (128, 512)
    kern2 = np.concatenate([kern[0:128], kern[128:256]], 1)   # (128, 16)
    # whb: (128, 16*512) bf16; col-block (d*2+oh) = Wh3[d, oh*128:(oh+1)*128, :]
    Wh3 = Wh.reshape(D, ORDER, HIDDEN)
    wh_blocks = [Wh3[d_i, oh * 128:(oh + 1) * 128, :]
                 for d_i in range(D) for oh in range(2)]
    whb = np.concatenate(wh_blocks, 1).astype(ml_dtypes.bfloat16)
    bhr = np.ascontiguousarray(bh.reshape(4, 128).T, dtype=np.float32)  # (128,4)
    ident = np.eye(128, dtype=np.float32)
    return {
        "gk0": f32(gk0), "gk1": f32(gk1), "augsm": f32(augsm),
        "pflip": f32(PflipAll), "a128p": f32(a128p),
        "kern2": f32(kern2), "whb": whb, "bhr": bhr, "ident": ident,
        "zeros32": np.zeros((128, 32), np.float32),
    }


def _emit(tc, nc, dram):
    """Emit the per-core program into the TileContext."""
    x_d, y_d = dram["x"], dram["y"]
    f32r = lambda ap: ap.bitcast(F32R)

    with (
        tc.tile_pool(name="const", bufs=1) as cst,
        tc.tile_pool(name="ring", bufs=9) as ring,
        tc.tile_pool(name="gw", bufs=2) as gw,
        tc.tile_pool(name="gw2", bufs=2) as gw2,
        tc.tile_pool(name="ustg", bufs=12) as ustg,
    ):
        # ---- resident constants -------------------------------------------
        ident = cst.tile([128, 128], F32)
        nc.sync.dma_start(ident[:], dram["ident"][:])
        kern_sb = cst.tile([128, 16], F32)
        nc.sync.dma_start(kern_sb[:], dram["kern2"][:])
        wh_sb = cst.tile([128, 16 * 512], BF16)
        nc.sync.dma_start(wh_sb[:], dram["whb"][:])
        bh_sb = cst.tile([128, 4], F32)
        nc.sync.dma_start(bh_sb[:], dram["bhr"][:])
        a8k0 = cst.tile([128, 256], F32)
        nc.sync.dma_start(a8k0[:].bitcast(F32R), dram["gk0"][:, 8 * 256:9 * 256].bitcast(F32R))
        a8k1 = cst.tile([128, 256], F32)
        nc.sync.dma_start(a8k1[:].bitcast(F32R), dram["gk1"][:, 8 * 256:9 * 256].bitcast(F32R))
        a8sm = cst.tile([8, 256], F32)
        nc.sync.dma_start(a8sm[:].bitcast(F32R), dram["augsm"][:, 8 * 256:9 * 256].bitcast(F32R))
        pflip = cst.tile([128, 256], F32)
        nc.sync.dma_start(pflip[:], dram["pflip"][:])
        a128 = cst.tile([128, 512], F32)
        nc.sync.dma_start(a128[:], dram["a128p"][:])

        ut = cst.tile([128, CBD], F32)          # U^T[j, (c,b,d)]
        seed = cst.tile([128, 512], F32)        # [p-half kt] in col-block kt
        ms0 = cst.tile([128, NCH, B_L, D, C], BF16)
        ms1 = cst.tile([128, NCH, B_L, D, C], BF16)

        # ---- head: U = x @ kernel, chunk carries --------------------------
        with (
            tc.tile_pool(name="xin", bufs=2) as xin,
            tc.tile_pool(name="xt", bufs=4) as xt,
            tc.tile_pool(name="lcp", bufs=2) as lcp,
            tc.tile_pool(name="mend", bufs=2) as mendp,
            tc.tile_pool(name="ps_head", bufs=3, space="PSUM") as ps_head,
            tc.tile_pool(name="ps_u", bufs=2, space="PSUM") as ps_u,
            tc.tile_pool(name="ps_lm", bufs=1, space="PSUM") as ps_lm,
        ):
            mend_prev = None
            for c in range(NCH):
                for b in range(B_L):
                    x_t = xin.tile([128, 256], F32)
                    nc.sync.dma_start(x_t[:], x_d[b, c * C:(c + 1) * C, :])
                    xT = []
                    for ih in range(2):
                        tp = ps_head.tile([128, 128], F32)
                        nc.tensor.transpose(tp[:], x_t[:, ih * 128:(ih + 1) * 128], ident[:])
                        xt_t = xt.tile([128, 128], F32)
                        if ih == 0:
                            nc.vector.tensor_copy(xt_t[:], tp[:])
                        else:
                            nc.scalar.copy(xt_t[:], tp[:])
                        xT.append(xt_t)
                    ups = ps_u.tile([128, 8], F32)
                    for ih in range(2):
                        nc.tensor.matmul(ups[:], xT[ih][:], kern_sb[:, ih * 8:(ih + 1) * 8],
                                         start=(ih == 0), stop=(ih == 1))
                    nc.vector.tensor_copy(ut[:, (c * B_L + b) * D:(c * B_L + b + 1) * D].bitcast(F32R), ups[:])

                # SEED[:, kt-block, c] = m_end_{c-1}
                for kt in range(2):
                    dst = seed[:, kt * 256 + c * BD: kt * 256 + (c + 1) * BD]
                    if c == 0:
                        nc.sync.dma_start(dst.bitcast(F32R), dram["zeros32"][:].bitcast(F32R))
                    else:
                        nc.vector.tensor_copy(dst.bitcast(F32R), mend_prev[:, kt * BD:(kt + 1) * BD])
                # L_c
                lps = ps_lm.tile([128, 2 * BD], F32)
                for oh in range(2):
                    nc.tensor.matmul(lps[:, oh * BD:(oh + 1) * BD],
                                     pflip[:, oh * 128:(oh + 1) * 128],
                                     ut[:, c * BD:(c + 1) * BD], start=True, stop=True)
                lc_sb = lcp.tile([128, 2 * BD], F32)
                nc.vector.tensor_copy(lc_sb[:], lps[:])
                if c == 0:
                    mend_prev = lc_sb
                else:
                    mps = ps_lm.tile([128, 2 * BD], F32)
                    for oh in range(2):
                        for kt in range(2):
                            nc.tensor.matmul(mps[:, oh * BD:(oh + 1) * BD],
                                             a128[:, kt * 256 + oh * 128: kt * 256 + (oh + 1) * 128],
                                             mend_prev[:, kt * BD:(kt + 1) * BD],
                                             start=(kt == 0), stop=(kt == 1))
                    mend_t = mendp.tile([128, 2 * BD], F32)
                    nc.vector.tensor_add(mend_t[:], mps[:], lc_sb[:])
                    mend_prev = mend_t

        # ---- scan ----------------------------------------------------------
        with tc.tile_pool(name="ps_scan", bufs=6, space="PSUM") as ps_scan:
            ring_hist = {}
            for j in range(C):
                ps = ps_scan.tile([128, 512], F32)
                if j < R:
                    g_t = gw.tile([128, 512], F32)
                    nc.sync.dma_start(g_t[:, 0:256].bitcast(F32R), dram["gk0"][:, j * 256:(j + 1) * 256].bitcast(F32R))
                    nc.sync.dma_start(g_t[:, 256:512].bitcast(F32R), dram["gk1"][:, j * 256:(j + 1) * 256].bitcast(F32R))
                    g2_t = gw2.tile([8, 256], F32)
                    nc.sync.dma_start(g2_t[:].bitcast(F32R), dram["augsm"][:, j * 256:(j + 1) * 256].bitcast(F32R))
                    for oh in range(2):
                        po = ps[:, oh * 256:(oh + 1) * 256]
                        nc.tensor.matmul(po, f32r(g_t[:, oh * 128:(oh + 1) * 128]),
                                         f32r(seed[:, 0:256]), start=True, stop=False)
                        nc.tensor.matmul(po, f32r(g_t[:, 256 + oh * 128:256 + (oh + 1) * 128]),
                                         f32r(seed[:, 256:512]), start=False, stop=False)
                        nc.tensor.matmul(po, f32r(g2_t[:, oh * 128:(oh + 1) * 128]),
                                         f32r(ut[0:8, :]), start=False, stop=True)
                else:
                    prev = ring_hist.pop(j - R)
                    # matmul rhs must start at partition 0/32/64 — stage the
                    # 8 injection rows down to partition 0 via DMA
                    stg = ustg.tile([8, CBD], F32)
                    nc.sync.dma_start(stg[:].bitcast(F32R), ut[j - 7:j + 1, :].bitcast(F32R))
                    for oh in range(2):
                        po = ps[:, oh * 256:(oh + 1) * 256]
                        nc.tensor.matmul(po, f32r(a8k0[:, oh * 128:(oh + 1) * 128]),
                                         f32r(prev[:, 0:256]), start=True, stop=False)
                        nc.tensor.matmul(po, f32r(a8k1[:, oh * 128:(oh + 1) * 128]),
                                         f32r(prev[:, 256:512]), start=False, stop=False)
                        nc.tensor.matmul(po, f32r(a8sm[:, oh * 128:(oh + 1) * 128]),
                                         f32r(stg[:]), start=False, stop=True)
                ps0 = ps[:, 0:256].rearrange("p (c b d) -> p c b d", c=NCH, b=B_L, d=D)
                ps1 = ps[:, 256:512].rearrange("p (c b d) -> p c b d", c=NCH, b=B_L, d=D)
                if j + R < C:
                    r_t = ring.tile([128, 512], F32)
                    nc.vector.tensor_copy(r_t[:].bitcast(F32R), ps[:])
                    ring_hist[j] = r_t
                nc.scalar.copy(ms0[:, :, :, :, j], ps0)
                nc.scalar.copy(ms1[:, :, :, :, j], ps1)

        # ---- dense + tanh + store -----------------------------------------
        with (
            tc.tile_pool(name="ps_dense", bufs=8, space="PSUM") as ps_dense,
            tc.tile_pool(name="ys", bufs=3) as ys,
        ):
            for c in range(NCH):
                for ht in range(4):
                    dps = ps_dense.tile([128, 512], F32)
                    k = 0
                    for d_i in range(D):
                        for oh in range(2):
                            lhsT = wh_sb[:, (d_i * 2 + oh) * 512 + ht * 128:
                                         (d_i * 2 + oh) * 512 + (ht + 1) * 128]
                            ms = ms0 if oh == 0 else ms1
                            nc.tensor.matmul(dps[:], lhsT, ms[:, c, :, d_i, :],
                                             start=(k == 0), stop=(k == 15))
                            k += 1
                    y_t = ys.tile([128, B_L, C], F32)
                    nc.scalar.activation(y_t[:], dps[:].rearrange("p (b j) -> p b j", b=B_L),
                                         mybir.ActivationFunctionType.Tanh,
                                         bias=bh_sb[:, ht:ht + 1])
                    nc.sync.dma_start(y_d[ht * 128:(ht + 1) * 128, :, c * C:(c + 1) * C], y_t[:])


def build_nc(debug=False):
    nc = bacc.Bacc("TRN2", target_bir_lowering=False, debug=debug, num_devices=NCORE)
    dram = {}
    dram["x"] = nc.dram_tensor("x", [B_L, T, INPUT_D], F32, kind="ExternalInput").ap()
    specs = {
        "kern2": ([128, 16], F32), "whb": ([128, 16 * 512], BF16),
        "bhr": ([128, 4], F32), "gk0": ([128, 9 * 256], F32),
        "gk1": ([128, 9 * 256], F32), "augsm": ([8, 9 * 256], F32),
        "pflip": ([128, 256], F32), "a128p": ([128, 512], F32),
        "ident": ([128, 128], F32), "zeros32": ([128, 32], F32),
    }
    for name, (shape, dt) in specs.items():
        dram[name] = nc.dram_tensor(name, shape, dt, kind="ExternalInput").ap()
    dram["y"] = nc.dram_tensor("y", [HIDDEN, B_L, T], F32, kind="ExternalOutput").ap()
    with tile.TileContext(nc) as tc:
        _emit(tc, nc, dram)
    nc.compile()
    return nc


_NC_CACHE = {}


def kernel(x, kernel, Wh, bh, A, B):
    kern = kernel
    consts = _host_precompute(np.asarray(A), np.asarray(B), np.asarray(kern),
                              np.asarray(Wh), np.asarray(bh))
    if "nc" not in _NC_CACHE:
        _NC_CACHE["nc"] = build_nc(debug=False)
    nc = _NC_CACHE["nc"]
    x = np.ascontiguousarray(np.asarray(x, dtype=np.float32))
    in_maps = [dict(consts, x=x[i * B_L:(i + 1) * B_L]) for i in range(NCORE)]
    res = run_bass_kernel_spmd(nc, in_maps, list(range(NCORE))).results
    y = np.empty((B_SZ, T, HIDDEN), np.float32)
    for i in range(NCORE):
        y[i * B_L:(i + 1) * B_L] = np.transpose(res[i]["y"], (1, 2, 0))
    return y
